# revision 12
# baseline (speedup 1.0000x reference)
"""CrossAttention kernel for 8 Trainium2 NeuronCores.

Sharding (tensor-parallel heads x data-parallel batch):
  core c -> batch b = c // 4, head-group g = c % 4 (heads 4g..4g+3).
  Each core: slice Wq/Wk/Wv columns + Wo rows for its 4 heads, compute full
  attention for those heads on its batch, produce a PARTIAL output
  y_part = attn_heads @ Wo_rows  [2048, 1024]. Host sums the 4 partials per
  batch and adds bo.

Per-core kernel (all matmuls in float32r = full-rate PE):
  1. Transpose context/query [S,D] -> X^T [D,S] via PE-transpose.
  2. Projections: kT/qT [dh, S] head-stacked in pairs, V natural [S, dh]
     with an appended ones column (gives the softmax denominator for free).
  3. Attention per head-pair, per 512-wide q-block, streaming over 16
     k-blocks: scores^T (2 heads row-packed into one PE pass, separate PSUM
     banks) -> exp (ACT, 1/8 scale fused) -> PV accumulate in PSUM
     (row 64 = sum of exp = softmax denominator r).
  4. Normalize: 1/r for both heads broadcast across partitions with a K=2
     selector matmul; single full-width multiply.
  5. Output projection vs Wo row-slice, DMA partial result out.
"""

import sys

sys.path.insert(0, "/opt/trn_rl_repo")

import numpy as np

B, S, D = 2, 2048, 1024
H, DH = 16, 64
P = 128
HPC = 4          # heads per core
NPAIR = 2        # head pairs per core
KC = D // P      # 8 contraction chunks for projections
NSB = S // P     # 16 seq blocks of 128
NQB = 4          # q blocks of 512
QW = S // NQB    # 512
NG = 4           # context groups of 512
HD_C = HPC * DH  # 256 head dims per core

_CACHE = {}


def _build():
    from concourse import bacc, tile
    import concourse.mybir as mybir
    from concourse.masks import make_identity

    F32 = mybir.dt.float32
    F32R = mybir.dt.float32r

    nc = bacc.Bacc("TRN2", target_bir_lowering=False, debug=False)

    x_q = nc.dram_tensor("x_q", [S, D], F32, kind="ExternalInput")
    x_c = nc.dram_tensor("x_c", [S, D], F32, kind="ExternalInput")
    wq = nc.dram_tensor("wq", [D, HD_C], F32, kind="ExternalInput")
    wk = nc.dram_tensor("wk", [D, HD_C], F32, kind="ExternalInput")
    wv = nc.dram_tensor("wv", [D, HD_C], F32, kind="ExternalInput")
    wo = nc.dram_tensor("wo", [HD_C, D], F32, kind="ExternalInput")
    ones = nc.dram_tensor("ones", [P, 64], F32, kind="ExternalInput")
    y = nc.dram_tensor("y", [S, D], F32, kind="ExternalOutput")

    wq_r = wq.ap().rearrange("(po pi) f -> pi po f", pi=P).bitcast(F32R)
    wk_r = wk.ap().rearrange("(po pi) f -> pi po f", pi=P).bitcast(F32R)
    wv_r = wv.ap().rearrange("(po pi) f -> pi po f", pi=P).bitcast(F32R)
    wo_r = wo.ap().rearrange("(po pi) f -> pi po f", pi=P).bitcast(F32R)

    with tile.TileContext(nc) as tc:
        with tc.tile_pool(name="consts", bufs=1) as consts, \
             tc.tile_pool(name="wpool", bufs=1) as wpool, \
             tc.tile_pool(name="pers", bufs=1) as pers, \
             tc.tile_pool(name="xstage", bufs=4) as xstage, \
             tc.tile_pool(name="xtp", bufs=3) as xtp, \
             tc.tile_pool(name="qtp", bufs=4) as qtp, \
             tc.tile_pool(name="epool", bufs=4) as epool, \
             tc.tile_pool(name="ypool", bufs=3) as ypool, \
             tc.tile_pool(name="dpool", bufs=3) as dpool, \
             tc.tile_pool(name="psst", bufs=2, space="PSUM") as psst, \
             tc.tile_pool(name="psmisc", bufs=2, space="PSUM") as psmisc, \
             tc.tile_pool(name="psacc", bufs=2, space="PSUM") as psacc:

            ident = consts.tile([P, P], F32)
            make_identity(nc, ident)
            onescol = consts.tile([1, 64], F32R)
            wq_sb = wpool.tile([P, KC, HD_C], F32R)
            wk_sb = wpool.tile([P, KC, HD_C], F32R)
            wv_sb = wpool.tile([P, KC, HD_C], F32R)
            wo_sb = wpool.tile([P, NPAIR, D], F32R)
            kT = [[pers.tile([P, QW], F32R, name=f"kT{m}_{g}") for g in range(NG)]
                  for m in range(NPAIR)]
            # V for all 4 heads, per group: [s', k-block-in-group, head, dh+1]
            vones = [pers.tile([P, 4, HPC, DH + 1], F32R, name=f"vones{g}")
                     for g in range(NG)]
            nc.gpsimd.dma_start(out=wv_sb, in_=wv_r)
            nc.gpsimd.dma_start(out=wk_sb, in_=wk_r)
            nc.gpsimd.dma_start(out=wq_sb, in_=wq_r)
            for g in range(NG):
                nc.gpsimd.dma_start(
                    out=vones[g][:, :, :, DH:DH + 1],
                    in_=ones.ap()[:, 0:16].rearrange("p (a h o) -> p a h o", a=4, o=1).bitcast(F32R),
                )
            nc.gpsimd.dma_start(out=wo_sb, in_=wo_r)
            nc.gpsimd.dma_start(out=onescol, in_=ones.ap()[0:1, :].bitcast(F32R))

            def transpose_block(src_dram, blk, xt_tile, engines=("vector", "vector")):
                """DMA rows [blk*128, 128) of [S, D] source; transpose the 8
                d-chunks into xt_tile[:, c, off:off+128] via two 4-chunk
                batches, each drained by one engine copy."""
                off = (blk % 4) * P
                stg = xstage.tile([P, D], F32, tag="stg", name="stg")
                nc.sync.dma_start(out=stg, in_=src_dram.ap()[blk * P:(blk + 1) * P, :])
                for half in range(2):
                    ptb = psmisc.tile([P, 512], F32, tag="m", name="ptb")
                    for c4 in range(4):
                        c = half * 4 + c4
                        nc.tensor.transpose(
                            ptb[:, c4 * P:(c4 + 1) * P], stg[:, c * P:(c + 1) * P], ident[:])
                    src_v = ptb[:].rearrange("p (a b) -> p a b", a=4)
                    dst_v = xt_tile[:, half * 4:(half + 1) * 4, off:off + P]
                    if engines[half] == "act":
                        nc.scalar.copy(out=dst_v, in_=src_v)
                    else:
                        nc.vector.tensor_copy(out=dst_v, in_=src_v)

            # ---- context path: kT + V ----
            def ctx_group(g):
                xt_g = xtp.tile([P, KC, QW], F32R, tag="xt", name=f"xt_c{g}")
                for sb4 in range(4):
                    transpose_block(x_c, g * 4 + sb4, xt_g, engines=("act", "vector"))
                for sb4 in range(4):
                    blk = g * 4 + sb4
                    vps = psmisc.tile([P, HD_C], F32, tag="m", name=f"vps{blk}")
                    for c in range(KC):
                        nc.tensor.matmul(
                            vps[:],
                            xt_g[:, c, sb4 * P:(sb4 + 1) * P],
                            wv_sb[:, c, :],
                            start=(c == 0), stop=(c == KC - 1),
                        )
                    nc.vector.tensor_copy(
                        out=vones[g][:, sb4, :, 0:DH],
                        in_=vps[:].rearrange("p (h d) -> p h d", h=HPC),
                    )
                for m in range(NPAIR):
                    kps = psmisc.tile([P, QW], F32, tag="m", name=f"kps{g}_{m}")
                    for c in range(KC):
                        nc.tensor.matmul(
                            kps[:],
                            wk_sb[:, c, m * P:(m + 1) * P],
                            xt_g[:, c, :],
                            start=(c == 0), stop=(c == KC - 1),
                        )
                    nc.vector.tensor_copy(out=kT[m][g][:, :], in_=kps[:])

            # ---- query path + attention + output projection, per q-block ----
            qTs, stacks = {}, {}

            def qpath(qb):
                xt_q = xtp.tile([P, KC, QW], F32R, tag="xt", name=f"xt_q{qb}")
                for sb4 in range(4):
                    transpose_block(x_q, qb * 4 + sb4, xt_q)
                qT = []
                stack = []
                for m in range(NPAIR):
                    qT.append(qtp.tile([P, QW], F32R, tag="qT", name=f"qT{qb}_{m}"))
                    stack.append(qtp.tile([P, QW], F32R, tag="stack", name=f"stack{qb}_{m}"))
                for m in range(NPAIR):
                    qps = psmisc.tile([P, QW], F32, tag="m", name=f"qps{qb}_{m}")
                    for c in range(KC):
                        nc.tensor.matmul(
                            qps[:],
                            wq_sb[:, c, m * P:(m + 1) * P],
                            xt_q[:, c, :],
                            start=(c == 0), stop=(c == KC - 1),
                        )
                    nc.vector.tensor_copy(out=qT[m][:, :], in_=qps[:])
                qTs[qb], stacks[qb] = qT, stack

            def attention(qb):
                qT, stack = qTs[qb], stacks[qb]
                for pair in range(NPAIR):
                    acc = [
                        psacc.tile([DH + 1, QW], F32, tag="acc", name=f"acc{qb}_{pair}_{hh}")
                        for hh in range(2)
                    ]
                    for i in range(NSB):
                        sT = psst.tile([P, 2 * QW], F32, tag="s", name=f"sT{qb}_{pair}_{i}")
                        eT = epool.tile([P, 2 * QW], F32R, tag="eT", name=f"eT{qb}_{pair}_{i}")
                        kTg = kT[pair][i // 4]
                        ib = i % 4
                        nc.tensor.matmul(
                            sT[:, 0:QW],
                            kTg[0:64, ib * P:(ib + 1) * P],
                            qT[pair][0:64, :],
                            start=True, stop=True,
                        )
                        nc.tensor.matmul(
                            sT[:, QW:2 * QW],
                            kTg[64:128, ib * P:(ib + 1) * P],
                            qT[pair][64:128, :],
                            start=True, stop=True,
                        )
                        nc.scalar.activation(
                            out=eT[:], in_=sT[:],
                            func=mybir.ActivationFunctionType.Exp,
                            scale=float(DH) ** -0.5,
                        )
                        for hh in range(2):
                            nc.tensor.matmul(
                                acc[hh][:],
                                vones[i // 4][:, i % 4, 2 * pair + hh, :],
                                eT[:, hh * QW:(hh + 1) * QW],
                                start=(i == 0), stop=(i == NSB - 1),
                            )
                    # normalize both heads: K=1 ones matmul broadcasts 1/r
                    # across 64 partitions, then copy+multiply per head
                    with nc.allow_low_precision(reason="f32r softmax normalization"):
                        for hh in range(2):
                            rrow = dpool.tile([1, QW], F32R, tag="rrow", name=f"rr{qb}_{pair}_{hh}")
                            nc.vector.reciprocal(out=rrow[:], in_=acc[hh][DH:DH + 1, :])
                            rb = psacc.tile([64, QW], F32, tag="acc", name=f"rb{qb}_{pair}_{hh}")
                            nc.tensor.matmul(rb[:], onescol[:], rrow[:], start=True, stop=True)
                            dst = stack[pair][hh * 64:(hh + 1) * 64, :]
                            nc.vector.tensor_copy(out=dst, in_=acc[hh][0:DH, :])
                            nc.vector.tensor_mul(out=dst, in0=dst, in1=rb[:])

            def wo_proj(qb):
                stack = stacks[qb]
                for st in range(4):
                    rsl = slice((qb * 4 + st) * P, (qb * 4 + st + 1) * P)
                    ysb = ypool.tile([P, D], F32, tag="ysb", name=f"ysb{qb}_{st}")
                    for nb in range(2):
                        yps = psacc.tile([P, 512], F32, tag="acc", name=f"yps{qb}_{st}_{nb}")
                        for m in range(NPAIR):
                            nc.tensor.matmul(
                                yps[:],
                                stack[m][:, st * P:(st + 1) * P],
                                wo_sb[:, m, nb * 512:(nb + 1) * 512],
                                start=(m == 0), stop=(m == NPAIR - 1),
                            )
                        nc.vector.tensor_copy(out=ysb[:, nb * 512:(nb + 1) * 512], in_=yps[:])
                    nc.sync.dma_start(out=y.ap()[rsl, :], in_=ysb)

            # ---- emission schedule: qb0 query path early; Wo(qb) after
            # qpath(qb+1) so the next q-block's transposes flow during
            # attention while Wo fills PE gaps. ----
            ctx_group(0)
            ctx_group(1)
            ctx_group(2)
            qpath(0)
            ctx_group(3)
            for qb in range(NQB):
                attention(qb)
                if qb + 1 < NQB:
                    qpath(qb + 1)
                wo_proj(qb)

    nc.compile()
    return nc


def _get_nc():
    if "nc" not in _CACHE:
        _CACHE["nc"] = _build()
    return _CACHE["nc"]


def _make_in_maps(query, context, Wq, Wk, Wv, Wo):
    ones = np.ones((P, 64), np.float32)
    in_maps = []
    for c in range(8):
        b, g = c // 4, c % 4
        csl = slice(g * HD_C, (g + 1) * HD_C)
        in_maps.append({
            "x_q": np.ascontiguousarray(query[b]),
            "x_c": np.ascontiguousarray(context[b]),
            "wq": np.ascontiguousarray(Wq[:, csl]),
            "wk": np.ascontiguousarray(Wk[:, csl]),
            "wv": np.ascontiguousarray(Wv[:, csl]),
            "wo": np.ascontiguousarray(Wo[csl, :]),
            "ones": ones,
        })
    return in_maps


def kernel(query, context, Wq, Wk, Wv, Wo, bo):
    from concourse.bass_utils import run_bass_kernel_spmd

    query = np.asarray(query, dtype=np.float32)
    context = np.asarray(context, dtype=np.float32)
    Wq = np.asarray(Wq, dtype=np.float32)
    Wk = np.asarray(Wk, dtype=np.float32)
    Wv = np.asarray(Wv, dtype=np.float32)
    Wo = np.asarray(Wo, dtype=np.float32)
    bo = np.asarray(bo, dtype=np.float32)

    nc = _get_nc()
    in_maps = _make_in_maps(query, context, Wq, Wk, Wv, Wo)
    res = run_bass_kernel_spmd(nc, in_maps, core_ids=list(range(8)))
    out = np.zeros((B, S, D), np.float32)
    for c in range(8):
        out[c // 4] += res.results[c]["y"]
    out += bo[None, None, :]
    return out


# revision 21
# speedup vs baseline: 86.6696x; 86.6696x over previous
"""CrossAttention kernel for 8 Trainium2 NeuronCores.

Sharding (tensor-parallel heads x data-parallel batch):
  core c -> batch b = c // 4, head-group g = c % 4 (heads 4g..4g+3).
  Each core: slice Wq/Wk/Wv columns + Wo rows for its 4 heads, compute full
  attention for those heads on its batch, produce a PARTIAL output
  y_part = attn_heads @ Wo_rows  [2048, 1024]. Host sums the 4 partials per
  batch and adds bo.

Per-core kernel (all matmuls in float32r = full-rate PE):
  1. Transpose context/query [S,D] -> X^T [D,S] via PE-transpose.
  2. Projections: kT/qT [dh, S] head-stacked in pairs, V natural [S, dh]
     with an appended ones column (gives the softmax denominator for free).
  3. Attention per head-pair, per 512-wide q-block, streaming over 16
     k-blocks: scores^T (2 heads row-packed into one PE pass, separate PSUM
     banks) -> exp (ACT, 1/8 scale fused) -> PV accumulate in PSUM
     (row 64 = sum of exp = softmax denominator r).
  4. Normalize: 1/r for both heads broadcast across partitions with a K=2
     selector matmul; single full-width multiply.
  5. Output projection vs Wo row-slice, DMA partial result out.
"""

import sys

sys.path.insert(0, "/opt/trn_rl_repo")

import numpy as np

B, S, D = 2, 2048, 1024
H, DH = 16, 64
P = 128
HPC = 4          # heads per core
NPAIR = 2        # head pairs per core
KC = D // P      # 8 contraction chunks for projections
NSB = S // P     # 16 seq blocks of 128
NQB = 4          # q blocks of 512
QW = S // NQB    # 512
NG = 4           # context groups of 512
HD_C = HPC * DH  # 256 head dims per core

_CACHE = {}


def _build():
    from concourse import bacc, tile
    import concourse.mybir as mybir

    F32 = mybir.dt.float32
    F32R = mybir.dt.float32r

    nc = bacc.Bacc("TRN2", target_bir_lowering=False, debug=False)

    x_q = nc.dram_tensor("x_q", [S, D], F32, kind="ExternalInput")
    x_c = nc.dram_tensor("x_c", [S, D], F32, kind="ExternalInput")
    wq = nc.dram_tensor("wq", [D, HD_C], F32, kind="ExternalInput")
    wk = nc.dram_tensor("wk", [D, HD_C], F32, kind="ExternalInput")
    wv = nc.dram_tensor("wv", [D, HD_C], F32, kind="ExternalInput")
    wo = nc.dram_tensor("wo", [HD_C, D], F32, kind="ExternalInput")
    ones = nc.dram_tensor("ones", [P, 64], F32, kind="ExternalInput")
    identity = nc.dram_tensor("identity", [P, P], F32, kind="ExternalInput")
    y = nc.dram_tensor("y", [S, D], F32, kind="ExternalOutput")

    wq_r = wq.ap().rearrange("(po pi) f -> pi po f", pi=P).bitcast(F32R)
    wk_r = wk.ap().rearrange("(po pi) f -> pi po f", pi=P).bitcast(F32R)
    wv_r = wv.ap().rearrange("(po pi) f -> pi po f", pi=P).bitcast(F32R)
    wo_r = wo.ap().rearrange("(po pi) f -> pi po f", pi=P).bitcast(F32R)

    with tile.TileContext(nc) as tc:
        with tc.tile_pool(name="consts", bufs=1) as consts, \
             tc.tile_pool(name="wpool", bufs=1) as wpool, \
             tc.tile_pool(name="pers", bufs=1) as pers, \
             tc.tile_pool(name="xstage", bufs=4) as xstage, \
             tc.tile_pool(name="xtp", bufs=3) as xtp, \
             tc.tile_pool(name="qtp", bufs=4) as qtp, \
             tc.tile_pool(name="epool", bufs=4) as epool, \
             tc.tile_pool(name="ypool", bufs=3) as ypool, \
             tc.tile_pool(name="dpool", bufs=3) as dpool, \
             tc.tile_pool(name="psst", bufs=2, space="PSUM") as psst, \
             tc.tile_pool(name="psmisc", bufs=2, space="PSUM") as psmisc, \
             tc.tile_pool(name="psacc", bufs=2, space="PSUM") as psacc:

            ident = consts.tile([P, P], F32R)
            nc.gpsimd.dma_start(out=ident, in_=identity.ap().bitcast(F32R))
            onescol = consts.tile([1, 64], F32R)
            wq_sb = wpool.tile([P, KC, HD_C], F32R)
            wk_sb = wpool.tile([P, KC, HD_C], F32R)
            wv_sb = wpool.tile([P, KC, HD_C], F32R)
            wo_sb = wpool.tile([P, NPAIR, D], F32R)
            kT = [[pers.tile([P, QW], F32R, name=f"kT{m}_{g}") for g in range(NG)]
                  for m in range(NPAIR)]
            # V for all 4 heads, per group: [s', k-block-in-group, head, dh+1]
            vones = [pers.tile([P, 4, HPC, DH + 1], F32R, name=f"vones{g}")
                     for g in range(NG)]
            nc.gpsimd.dma_start(out=wv_sb, in_=wv_r)
            nc.gpsimd.dma_start(out=wk_sb, in_=wk_r)
            nc.gpsimd.dma_start(out=wq_sb, in_=wq_r)
            for g in range(NG):
                nc.gpsimd.dma_start(
                    out=vones[g][:, :, :, DH:DH + 1],
                    in_=ones.ap()[:, 0:16].rearrange("p (a h o) -> p a h o", a=4, o=1).bitcast(F32R),
                )
            nc.gpsimd.dma_start(out=wo_sb, in_=wo_r)
            nc.gpsimd.dma_start(out=onescol, in_=ones.ap()[0:1, :].bitcast(F32R))

            def transpose_block(src_dram, blk, xt_tile, engines=("vector", "vector"),
                                ptb_pool=None):
                """DMA rows [blk*128, 128) of [S, D] source; transpose the 8
                d-chunks into xt_tile[:, c, off:off+128] via two 4-chunk
                batches, each drained by one engine copy."""
                off = (blk % 4) * P
                stg = xstage.tile([P, D], F32R, tag="stg", name="stg")
                nc.sync.dma_start(
                    out=stg, in_=src_dram.ap()[blk * P:(blk + 1) * P, :].bitcast(F32R))
                pool, ptag = ptb_pool or (psmisc, "m")
                for half in range(2):
                    ptb = pool.tile([P, 512], F32R, tag=ptag, name="ptb")
                    for c4 in range(4):
                        c = half * 4 + c4
                        nc.tensor.transpose(
                            ptb[:, c4 * P:(c4 + 1) * P], stg[:, c * P:(c + 1) * P], ident[:])
                    src_v = ptb[:].rearrange("p (a b) -> p a b", a=4)
                    dst_v = xt_tile[:, half * 4:(half + 1) * 4, off:off + P]
                    if engines[half] == "act":
                        nc.scalar.copy(out=dst_v, in_=src_v)
                    else:
                        nc.vector.tensor_copy(out=dst_v, in_=src_v)

            # ---- context path: kT + V ----
            def ctx_group(g):
                xt_g = xtp.tile([P, KC, QW], F32R, tag="xt", name=f"xt_c{g}")
                for sb4 in range(4):
                    transpose_block(x_c, g * 4 + sb4, xt_g, engines=("act", "vector"))
                for sb4 in range(4):
                    blk = g * 4 + sb4
                    vps = psmisc.tile([P, HD_C], F32, tag="m", name=f"vps{blk}")
                    for c in range(KC):
                        nc.tensor.matmul(
                            vps[:],
                            xt_g[:, c, sb4 * P:(sb4 + 1) * P],
                            wv_sb[:, c, :],
                            start=(c == 0), stop=(c == KC - 1),
                        )
                    nc.vector.tensor_copy(
                        out=vones[g][:, sb4, :, 0:DH],
                        in_=vps[:].rearrange("p (h d) -> p h d", h=HPC),
                    )
                for m in range(NPAIR):
                    kps = psmisc.tile([P, QW], F32, tag="m", name=f"kps{g}_{m}")
                    for c in range(KC):
                        nc.tensor.matmul(
                            kps[:],
                            wk_sb[:, c, m * P:(m + 1) * P],
                            xt_g[:, c, :],
                            start=(c == 0), stop=(c == KC - 1),
                        )
                    nc.vector.tensor_copy(out=kT[m][g][:, :], in_=kps[:])

            # ---- query path + attention + output projection, per q-block ----
            qTs, stacks = {}, {}

            def qpath(qb):
                xt_q = xtp.tile([P, KC, QW], F32R, tag="xt", name=f"xt_q{qb}")
                for sb4 in range(4):
                    transpose_block(x_q, qb * 4 + sb4, xt_q)
                qT = []
                stack = []
                for m in range(NPAIR):
                    qT.append(qtp.tile([P, QW], F32R, tag="qT", name=f"qT{qb}_{m}"))
                    stack.append(qtp.tile([P, QW], F32R, tag="stack", name=f"stack{qb}_{m}"))
                for m in range(NPAIR):
                    qps = psmisc.tile([P, QW], F32, tag="m", name=f"qps{qb}_{m}")
                    for c in range(KC):
                        nc.tensor.matmul(
                            qps[:],
                            wq_sb[:, c, m * P:(m + 1) * P],
                            xt_q[:, c, :],
                            start=(c == 0), stop=(c == KC - 1),
                        )
                    nc.vector.tensor_copy(out=qT[m][:, :], in_=qps[:])
                qTs[qb], stacks[qb] = qT, stack

            def attention(qb):
                qT, stack = qTs[qb], stacks[qb]
                for pair in range(NPAIR):
                    acc = [
                        psacc.tile([DH + 1, QW], F32, tag="acc", name=f"acc{qb}_{pair}_{hh}")
                        for hh in range(2)
                    ]
                    for i in range(NSB):
                        sT = psst.tile([P, 2 * QW], F32, tag="s", name=f"sT{qb}_{pair}_{i}")
                        eT = epool.tile([P, 2 * QW], F32R, tag="eT", name=f"eT{qb}_{pair}_{i}")
                        kTg = kT[pair][i // 4]
                        ib = i % 4
                        nc.tensor.matmul(
                            sT[:, 0:QW],
                            kTg[0:64, ib * P:(ib + 1) * P],
                            qT[pair][0:64, :],
                            start=True, stop=True,
                        )
                        nc.tensor.matmul(
                            sT[:, QW:2 * QW],
                            kTg[64:128, ib * P:(ib + 1) * P],
                            qT[pair][64:128, :],
                            start=True, stop=True,
                        )
                        nc.scalar.activation(
                            out=eT[:], in_=sT[:],
                            func=mybir.ActivationFunctionType.Exp,
                            scale=float(DH) ** -0.5,
                        )
                        for hh in range(2):
                            nc.tensor.matmul(
                                acc[hh][:],
                                vones[i // 4][:, i % 4, 2 * pair + hh, :],
                                eT[:, hh * QW:(hh + 1) * QW],
                                start=(i == 0), stop=(i == NSB - 1),
                            )
                    # normalize both heads: K=1 ones matmul broadcasts 1/r
                    # across 64 partitions, then copy+multiply per head
                    with nc.allow_low_precision(reason="f32r softmax normalization"):
                        for hh in range(2):
                            rrow = dpool.tile([1, QW], F32R, tag="rrow", name=f"rr{qb}_{pair}_{hh}")
                            nc.vector.reciprocal(out=rrow[:], in_=acc[hh][DH:DH + 1, :])
                            rb = psacc.tile([64, QW], F32, tag="acc", name=f"rb{qb}_{pair}_{hh}")
                            nc.tensor.matmul(rb[:], onescol[:], rrow[:], start=True, stop=True)
                            dst = stack[pair][hh * 64:(hh + 1) * 64, :]
                            nc.vector.tensor_copy(out=dst, in_=acc[hh][0:DH, :])
                            nc.vector.tensor_mul(out=dst, in0=dst, in1=rb[:])

            def wo_proj(qb):
                stack = stacks[qb]
                for st in range(4):
                    rsl = slice((qb * 4 + st) * P, (qb * 4 + st + 1) * P)
                    ysb = ypool.tile([P, D], F32, tag="ysb", name=f"ysb{qb}_{st}")
                    for nb in range(2):
                        yps = psacc.tile([P, 512], F32, tag="acc", name=f"yps{qb}_{st}_{nb}")
                        for m in range(NPAIR):
                            nc.tensor.matmul(
                                yps[:],
                                stack[m][:, st * P:(st + 1) * P],
                                wo_sb[:, m, nb * 512:(nb + 1) * 512],
                                start=(m == 0), stop=(m == NPAIR - 1),
                            )
                        nc.vector.tensor_copy(out=ysb[:, nb * 512:(nb + 1) * 512], in_=yps[:])
                    nc.sync.dma_start(out=y.ap()[rsl, :], in_=ysb)

            # ---- emission schedule: qb0 query path early; Wo(qb) after
            # qpath(qb+1) so the next q-block's transposes flow during
            # attention while Wo fills PE gaps. ----
            ctx_group(0)
            ctx_group(1)
            qpath(0)
            ctx_group(2)
            ctx_group(3)
            for qb in range(NQB):
                attention(qb)
                if qb + 1 < NQB:
                    qpath(qb + 1)
                wo_proj(qb)

    nc.compile()
    return nc


def _get_nc():
    if "nc" not in _CACHE:
        _CACHE["nc"] = _build()
    return _CACHE["nc"]


def _make_in_maps(query, context, Wq, Wk, Wv, Wo):
    ones = np.ones((P, 64), np.float32)
    ident = np.eye(P, dtype=np.float32)
    in_maps = []
    for c in range(8):
        b, g = c // 4, c % 4
        csl = slice(g * HD_C, (g + 1) * HD_C)
        in_maps.append({
            "x_q": np.ascontiguousarray(query[b]),
            "x_c": np.ascontiguousarray(context[b]),
            "wq": np.ascontiguousarray(Wq[:, csl]),
            "wk": np.ascontiguousarray(Wk[:, csl]),
            "wv": np.ascontiguousarray(Wv[:, csl]),
            "wo": np.ascontiguousarray(Wo[csl, :]),
            "ones": ones,
            "identity": ident,
        })
    return in_maps


def kernel(query, context, Wq, Wk, Wv, Wo, bo):
    from concourse.bass_utils import run_bass_kernel_spmd

    query = np.asarray(query, dtype=np.float32)
    context = np.asarray(context, dtype=np.float32)
    Wq = np.asarray(Wq, dtype=np.float32)
    Wk = np.asarray(Wk, dtype=np.float32)
    Wv = np.asarray(Wv, dtype=np.float32)
    Wo = np.asarray(Wo, dtype=np.float32)
    bo = np.asarray(bo, dtype=np.float32)

    nc = _get_nc()
    in_maps = _make_in_maps(query, context, Wq, Wk, Wv, Wo)
    res = run_bass_kernel_spmd(nc, in_maps, core_ids=list(range(8)))
    out = np.zeros((B, S, D), np.float32)
    for c in range(8):
        out[c // 4] += res.results[c]["y"]
    out += bo[None, None, :]
    return out
